# revision 3
# baseline (speedup 1.0000x reference)
"""Trainium2 Bass kernel for nn_DataTermLayer (data-term update of optical flow).

Math: the reference's bilinear warp feeds *normalized* coords in [-1,1] into a
pixel-space sampler, so the gather only touches I1[b, 0:3, 0:3] and the warp
value is piecewise-bilinear in nx = (2w+u)/511 - 1, ny = (2h+v)/511 - 1:

  warp = [nx>=0][ny>=0] * bilin(P, nx, ny)
  out_u = u + (0.1*I2 - 0.1*warp) * (I1[h+1,w]-I1[h,w])
  out_v = v + (0.1*I2 - 0.1*warp) * (I1[h,w+1]-I1[h,w])

This version computes everything in bf16 (rel-err budget 2e-2; measured
~1e-3) and drops the x>1/y>1 hat-basis correction terms (they matter only in
the last row/col, error ~1e-4): warp == A0 + B0*nx + ny*(A1 + B1*nx), with
per-image coefficients folded with -0.1 on the host and applied as ACT
scale/bias APs.  Masks are exact and applied only on the ambiguous strips
(cols 252..258, rows 128..255 of rb1 + rows 256..258), everywhere else the
mask value is provably constant given |flow| < 6.

Layout per image: [p=row%128, rb=row//128, w].  I1 row-shift comes from an
SBUF->SBUF partition-shift DMA (no second HBM read).  Inputs arrive as
f32->bf16 cast DMAs (SWDGE).  Output is stored bf16 and upcast on host.

Engine budget rule (measured): DVE bf16 dense TT = 0.59 ns/elem (2x mode),
TS = 0.33 (4x), strided ~1.25; ACT = 0.98 always; Pool poisons DVE when
running elementwise concurrently, so Pool only does memsets + SWDGE issue.

Sharding: pure data-parallel, 4 images per core across 8 cores.
"""
import sys

sys.path.insert(0, "/opt/trn_rl_repo")

import numpy as np
import ml_dtypes

import concourse.bass as bass
import concourse.mybir as mybir
from concourse.bass_utils import run_bass_kernel_spmd
from concourse.tile import TileContext

F32 = mybir.dt.float32
BF16 = mybir.dt.bfloat16
ALU = mybir.AluOpType
ACTF = mybir.ActivationFunctionType

R511 = float(np.float32(1.0) / np.float32(511.0))
WZ = 252          # first possibly-warped column (2*251 + umax < 511)
NW = 512 - WZ     # warp-region columns
XSTRIP = 7        # cols WZ..WZ+6 have an ambiguous x-mask (2w+u vs 511)


def build_nc(n_imgs: int = 4):
    W, H, NRB = 512, 512, 4
    FD = NRB * W          # free elems per partition for one [512,512] image
    nc = bass.Bass()

    I1 = nc.dram_tensor("I1", [n_imgs, H, W], F32, kind="ExternalInput")
    I2 = nc.dram_tensor("I2", [n_imgs, H, W], F32, kind="ExternalInput")
    FL = nc.dram_tensor("FL", [n_imgs, H, W, 2], F32, kind="ExternalInput")
    # gx: (2w)*r511 - 1 for w in [WZ,512), tiled x3 rb; bf16
    GX = nc.dram_tensor("GX", [128, 3 * NW], BF16, kind="ExternalInput")
    # per-partition f32 consts: col rb: (2*(128*rb+p))*r511 - 1; cols 4..:
    # per-image warp coeffs (-0.1-scaled): 4*b+{0:A0,1:B0,2:A1,3:B1}
    CC = nc.dram_tensor("CC", [128, 4 + 4 * n_imgs], F32, kind="ExternalInput")
    OUT = nc.dram_tensor("OUT", [n_imgs, H, W, 2], BF16, kind="ExternalOutput")

    with TileContext(nc) as tc:
        with (
            tc.tile_pool(name="stat", bufs=1) as pstat,
            tc.tile_pool(name="pin", bufs=3) as pin,
            tc.tile_pool(name="ptmp", bufs=2) as ptmp,
        ):
            gx = pstat.tile([128, 3 * NW], BF16)
            nc.sync.dma_start(gx[:], GX[:])
            gx3 = gx[:].rearrange("p (rb w) -> p rb w", rb=3)
            cc = pstat.tile([128, 4 + 4 * n_imgs], F32)
            nc.sync.dma_start(cc[:], CC[:])

            def cC(j):
                return cc[:, j : j + 1]

            for b in range(n_imgs):
                # ---------------- loads (f32 -> bf16 cast via SWDGE) --------
                i1 = pin.tile([128, FD], BF16, tag="i1")
                i1v = i1[:].rearrange("p (rb w) -> p rb w", rb=NRB)
                nc.gpsimd.dma_start(
                    i1v, I1[b].rearrange("(rb p) w -> p rb w", p=128)
                )
                i2 = pin.tile([128, FD], BF16, tag="i2")
                i2v = i2[:].rearrange("p (rb w) -> p rb w", rb=NRB)
                nc.gpsimd.dma_start(
                    i2v, I2[b].rearrange("(rb p) w -> p rb w", p=128)
                )
                fl = pin.tile([128, FD * 2], BF16, tag="fl")
                flv = fl[:].rearrange("p (rb w c) -> p rb w c", rb=NRB, c=2)
                nc.gpsimd.dma_start(
                    flv, FL[b].rearrange("(rb p) w c -> p rb w c", p=128)
                )

                # ---------------- i1 shifted down one row (SBUF->SBUF) ------
                i1r = pin.tile([128, FD], BF16, tag="i1r")
                i1rv = i1r[:].rearrange("p (rb w) -> p rb w", rb=NRB)
                nc.sync.dma_start(i1rv[0:127], i1v[1:128])
                nc.sync.dma_start(i1rv[127:128, 0:3], i1v[0:1, 1:4])
                nc.sync.dma_start(i1rv[127:128, 3:4], i1v[127:128, 3:4])

                # ---------------- gradients --------------------------------
                g1 = ptmp.tile([128, FD], BF16, tag="g1")
                nc.vector.tensor_tensor(g1[:], i1r[:], i1[:], ALU.subtract)
                g2 = ptmp.tile([128, FD], BF16, tag="g2")
                g2v = g2[:].rearrange("p (rb w) -> p rb w", rb=NRB)
                nc.vector.tensor_tensor(
                    g2v[:, :, 0:511], i1v[:, :, 1:512], i1v[:, :, 0:511],
                    ALU.subtract,
                )
                nc.gpsimd.memset(g2v[:, :, 511:512], 0.0)

                # ---------------- dt = 0.1*I2 (- 0.1*warp on warp region) ---
                dt = ptmp.tile([128, FD], BF16, tag="dt")
                nc.scalar.activation(dt[:], i2[:], ACTF.Copy, bias=0.0, scale=0.1)
                dtv = dt[:].rearrange("p (rb w) -> p rb w", rb=NRB)

                # warp region: rb 1..3, cols WZ..511  ([128, 3, NW])
                uw = ptmp.tile([128, 3, NW], BF16, tag="uw")
                nc.scalar.activation(
                    uw[:], flv[:, 1:4, WZ:, 0], ACTF.Copy, bias=0.0, scale=R511
                )
                x = ptmp.tile([128, 3, NW], BF16, tag="x")
                nc.vector.tensor_tensor(x[:], uw[:], gx3[:], ALU.add)
                y = ptmp.tile([128, 3, NW], BF16, tag="y")
                for rb in range(1, 4):
                    nc.scalar.activation(
                        y[:, rb - 1, :], flv[:, rb, WZ:, 1], ACTF.Identity,
                        bias=cC(rb), scale=R511,
                    )
                t0 = ptmp.tile([128, 3, NW], BF16, tag="t0")
                nc.scalar.activation(
                    t0[:], x[:], ACTF.Identity,
                    bias=cC(4 + 4 * b + 0), scale=cC(4 + 4 * b + 1),
                )
                t1 = ptmp.tile([128, 3, NW], BF16, tag="t1")
                nc.scalar.activation(
                    t1[:], x[:], ACTF.Identity,
                    bias=cC(4 + 4 * b + 2), scale=cC(4 + 4 * b + 3),
                )
                nc.vector.tensor_tensor(t1[:], y[:], t1[:], ALU.mult)
                nc.vector.tensor_tensor(t0[:], t0[:], t1[:], ALU.add)
                # masks (exact) on the ambiguous strips only
                nc.vector.scalar_tensor_tensor(
                    t0[:, :, 0:XSTRIP], x[:, :, 0:XSTRIP], 0.0,
                    t0[:, :, 0:XSTRIP], ALU.is_ge, ALU.mult,
                )
                nc.vector.scalar_tensor_tensor(
                    t0[:, 0:1, :], y[:, 0:1, :], 0.0, t0[:, 0:1, :],
                    ALU.is_ge, ALU.mult,
                )
                nc.vector.scalar_tensor_tensor(
                    t0[0:3, 1:2, :], y[0:3, 1:2, :], 0.0, t0[0:3, 1:2, :],
                    ALU.is_ge, ALU.mult,
                )
                nc.vector.tensor_tensor(
                    dtv[:, 1:4, WZ:], dtv[:, 1:4, WZ:], t0[:], ALU.add
                )

                # ---------------- flow update ------------------------------
                corr = ptmp.tile([128, FD * 2], BF16, tag="corr")
                corr2 = corr[:].rearrange("p (q c) -> p q c", c=2)
                nc.vector.tensor_tensor(corr2[:, :, 0], dt[:], g1[:], ALU.mult)
                nc.vector.tensor_tensor(corr2[:, :, 1], dt[:], g2[:], ALU.mult)
                nc.vector.tensor_tensor(fl[:], fl[:], corr[:], ALU.add)

                nc.sync.dma_start(
                    OUT[b].rearrange("(rb p) w c -> p rb w c", p=128), flv
                )
    legalize_single_wait(nc)
    return nc


# ---------------------------------------------------------------------------
# Post-pass: this walrus build encodes a single sync-wait slot per TPB
# instruction. Tile's sem assignment can emit 2+ waits on one instruction;
# hoist all but the last wait onto same-engine EventSemaphore carriers placed
# immediately before it (the sequencer then waits sequentially, which is
# semantically identical).
def legalize_single_wait(nc):
    import bass_rust

    capped = {
        mybir.EngineType.Activation,
        mybir.EngineType.DVE,
        mybir.EngineType.Pool,
        mybir.EngineType.PE,
        mybir.EngineType.SP,
    }
    exempt = {"EventSemaphore", "NoOp", "TriggerDma"}
    n = 0
    for fn in nc.m.functions:
        for blk in fn.blocks:
            insts = blk.instructions  # live list
            rebuilt = []
            changed = False
            for inst in list(insts):
                si = inst.sync_info
                waits = list(si.on_wait) if si is not None else []
                if (
                    len(waits) > 1
                    and inst.engine in capped
                    and str(inst.opcode) not in exempt
                ):
                    for w in waits[:-1]:
                        ev = mybir.InstEventSemaphore(
                            name=f"waitcarrier_{inst.name}_{n}", ins=[], outs=[]
                        )
                        ev.engine = inst.engine
                        ev.sync_info = bass_rust.SyncInfo(
                            on_wait=[w], on_update=[]
                        )
                        rebuilt.append(ev)
                        n += 1
                    inst.sync_info = bass_rust.SyncInfo(
                        on_wait=[waits[-1]], on_update=list(si.on_update)
                    )
                    changed = True
                rebuilt.append(inst)
            if changed:
                insts[:] = rebuilt
    return n


def host_consts(I1c: np.ndarray, n_imgs: int) -> np.ndarray:
    """[128, 4 + 4*n] f32: col rb: (2*(128*rb+p))*r511 - 1; then per-image
    -0.1-scaled bilinear coeffs A0,B0,A1,B1 of the corner patch."""
    f = np.float32
    cc = np.zeros((128, 4 + 4 * n_imgs), dtype=np.float32)
    p = np.arange(128, dtype=np.float64)
    for rb in range(4):
        cc[:, rb] = (2.0 * (128.0 * rb + p)) / 511.0 - 1.0
    for b in range(n_imgs):
        P = I1c[b, 0:3, 0:3].astype(np.float64)
        a0 = P[0, 0]
        b0 = P[0, 1] - P[0, 0]
        a1 = P[1, 0] - P[0, 0]
        b1 = P[1, 1] - P[1, 0] - P[0, 1] + P[0, 0]
        cc[:, 4 + 4 * b + 0] = f(-0.1 * a0)
        cc[:, 4 + 4 * b + 1] = f(-0.1 * b0)
        cc[:, 4 + 4 * b + 2] = f(-0.1 * a1)
        cc[:, 4 + 4 * b + 3] = f(-0.1 * b1)
    return cc


def host_gx() -> np.ndarray:
    w = np.arange(WZ, 512, dtype=np.float64)
    gx = (2.0 * w) / 511.0 - 1.0
    return np.tile(gx.astype(ml_dtypes.bfloat16), (128, 3))


_NC = None


def _get_nc():
    global _NC
    if _NC is None:
        _NC = build_nc(4)
    return _NC


def run(I1, I2, flow, trace=False, **kw):
    I1 = np.asarray(I1)
    I2 = np.asarray(I2)
    flow = np.asarray(flow)
    assert float(np.abs(flow).max()) < 6.0, "flow magnitude exceeds mask-strip bound"
    nc = _get_nc()
    gx = host_gx()
    in_maps = []
    per = I1.shape[0] // 8
    for c in range(8):
        sl = slice(c * per, (c + 1) * per)
        i1c = np.ascontiguousarray(I1[sl, :, :, 0], dtype=np.float32)
        in_maps.append(
            {
                "I1": i1c,
                "I2": np.ascontiguousarray(I2[sl, :, :, 0], dtype=np.float32),
                "FL": np.ascontiguousarray(flow[sl], dtype=np.float32),
                "GX": gx,
                "CC": host_consts(i1c, per),
            }
        )
    res = run_bass_kernel_spmd(nc, in_maps, list(range(8)), trace=trace, **kw)
    out = np.concatenate(
        [r["OUT"].astype(np.float32) for r in res.results], axis=0
    )
    return out, res


def kernel(I1, I2, flow):
    out, _ = run(I1, I2, flow)
    return out.astype(np.float32)


# revision 4
# speedup vs baseline: 1.0069x; 1.0069x over previous
"""Trainium2 Bass kernel for nn_DataTermLayer (data-term update of optical flow).

Math: the reference's bilinear warp feeds *normalized* coords in [-1,1] into a
pixel-space sampler, so the gather only touches I1[b, 0:3, 0:3] and the warp
value is piecewise-bilinear in nx = (2w+u)/511 - 1, ny = (2h+v)/511 - 1:

  warp = [nx>=0][ny>=0] * bilin(P, nx, ny)
  out_u = u + (0.1*I2 - 0.1*warp) * (I1[h+1,w]-I1[h,w])
  out_v = v + (0.1*I2 - 0.1*warp) * (I1[h,w+1]-I1[h,w])

This version computes everything in bf16 (rel-err budget 2e-2; measured
~1e-3) and drops the x>1/y>1 hat-basis correction terms (they matter only in
the last row/col, error ~1e-4): warp == A0 + B0*nx + ny*(A1 + B1*nx), with
per-image coefficients folded with -0.1 on the host and applied as ACT
scale/bias APs.  Masks are exact and applied only on the ambiguous strips
(cols 252..258, rows 128..255 of rb1 + rows 256..258), everywhere else the
mask value is provably constant given |flow| < 6.

Layout per image: [p=row%128, rb=row//128, w].  I1 row-shift comes from an
SBUF->SBUF partition-shift DMA (no second HBM read).  Inputs arrive as
f32->bf16 cast DMAs (SWDGE).  Output is stored bf16 and upcast on host.

Engine budget rule (measured): DVE bf16 dense TT = 0.59 ns/elem (2x mode),
TS = 0.33 (4x), strided ~1.25; ACT = 0.98 always; Pool poisons DVE when
running elementwise concurrently, so Pool only does memsets + SWDGE issue.

Sharding: pure data-parallel, 4 images per core across 8 cores.
"""
import sys

sys.path.insert(0, "/opt/trn_rl_repo")

import numpy as np
import ml_dtypes

import concourse.bass as bass
import concourse.mybir as mybir
from concourse.bass_utils import run_bass_kernel_spmd
from concourse.tile import TileContext

F32 = mybir.dt.float32
BF16 = mybir.dt.bfloat16
ALU = mybir.AluOpType
ACTF = mybir.ActivationFunctionType

R511 = float(np.float32(1.0) / np.float32(511.0))
WZ = 252          # first possibly-warped column (2*251 + umax < 511)
NW = 512 - WZ     # warp-region columns
XSTRIP = 7        # cols WZ..WZ+6 have an ambiguous x-mask (2w+u vs 511)


def build_nc(n_imgs: int = 4):
    W, H, NRB = 512, 512, 4
    FD = NRB * W          # free elems per partition for one [512,512] image
    nc = bass.Bass()

    I1 = nc.dram_tensor("I1", [n_imgs, H, W], F32, kind="ExternalInput")
    I2 = nc.dram_tensor("I2", [n_imgs, H, W], F32, kind="ExternalInput")
    FL = nc.dram_tensor("FL", [n_imgs, H, W, 2], F32, kind="ExternalInput")
    # gx: (2w)*r511 - 1 for w in [WZ,512), tiled x3 rb; bf16
    GX = nc.dram_tensor("GX", [128, 3 * NW], BF16, kind="ExternalInput")
    # per-partition f32 consts: col rb: (2*(128*rb+p))*r511 - 1; cols 4..:
    # per-image warp coeffs (-0.1-scaled): 4*b+{0:A0,1:B0,2:A1,3:B1}
    CC = nc.dram_tensor("CC", [128, 4 + 4 * n_imgs], F32, kind="ExternalInput")
    OUT = nc.dram_tensor("OUT", [n_imgs, H, W, 2], BF16, kind="ExternalOutput")

    with TileContext(nc) as tc:
        with (
            tc.tile_pool(name="stat", bufs=1) as pstat,
            tc.tile_pool(name="pin", bufs=3) as pin,
            tc.tile_pool(name="ptmp", bufs=2) as ptmp,
        ):
            gx = pstat.tile([128, 3 * NW], BF16)
            nc.sync.dma_start(gx[:], GX[:])
            gx3 = gx[:].rearrange("p (rb w) -> p rb w", rb=3)
            cc = pstat.tile([128, 4 + 4 * n_imgs], F32)
            nc.sync.dma_start(cc[:], CC[:])

            def cC(j):
                return cc[:, j : j + 1]

            for b in range(n_imgs):
                # ---------------- loads (f32 -> bf16 cast via SWDGE) --------
                i1 = pin.tile([128, FD], BF16, tag="i1")
                i1v = i1[:].rearrange("p (rb w) -> p rb w", rb=NRB)
                nc.gpsimd.dma_start(
                    i1v, I1[b].rearrange("(rb p) w -> p rb w", p=128)
                )
                i2 = pin.tile([128, FD], BF16, tag="i2")
                i2v = i2[:].rearrange("p (rb w) -> p rb w", rb=NRB)
                nc.gpsimd.dma_start(
                    i2v, I2[b].rearrange("(rb p) w -> p rb w", p=128)
                )
                fl = pin.tile([128, FD * 2], BF16, tag="fl")
                flv = fl[:].rearrange("p (rb w c) -> p rb w c", rb=NRB, c=2)
                nc.gpsimd.dma_start(
                    flv, FL[b].rearrange("(rb p) w c -> p rb w c", p=128)
                )

                # ---------------- i1 shifted down one row (SBUF->SBUF) ------
                i1r = pin.tile([128, FD], BF16, tag="i1r")
                i1rv = i1r[:].rearrange("p (rb w) -> p rb w", rb=NRB)
                nc.sync.dma_start(i1rv[0:127], i1v[1:128])
                nc.gpsimd.dma_start(i1rv[127:128, 0:3], i1v[0:1, 1:4])
                nc.gpsimd.dma_start(i1rv[127:128, 3:4], i1v[127:128, 3:4])

                # ---------------- gradients --------------------------------
                g1 = ptmp.tile([128, FD], BF16, tag="g1")
                nc.vector.tensor_tensor(g1[:], i1r[:], i1[:], ALU.subtract)
                g2 = ptmp.tile([128, FD], BF16, tag="g2")
                g2v = g2[:].rearrange("p (rb w) -> p rb w", rb=NRB)
                nc.vector.tensor_tensor(
                    g2v[:, :, 0:511], i1v[:, :, 1:512], i1v[:, :, 0:511],
                    ALU.subtract,
                )
                nc.gpsimd.memset(g2v[:, :, 511:512], 0.0)

                # ---------------- dt = 0.1*I2 (- 0.1*warp on warp region) ---
                dt = ptmp.tile([128, FD], BF16, tag="dt")
                nc.scalar.activation(dt[:], i2[:], ACTF.Copy, bias=0.0, scale=0.1)
                dtv = dt[:].rearrange("p (rb w) -> p rb w", rb=NRB)

                # warp region: rb 1..3, cols WZ..511  ([128, 3, NW])
                uw = ptmp.tile([128, 3, NW], BF16, tag="uw")
                nc.scalar.activation(
                    uw[:], flv[:, 1:4, WZ:, 0], ACTF.Copy, bias=0.0, scale=R511
                )
                x = ptmp.tile([128, 3, NW], BF16, tag="x")
                nc.vector.tensor_tensor(x[:], uw[:], gx3[:], ALU.add)
                y = ptmp.tile([128, 3, NW], BF16, tag="y")
                for rb in range(1, 4):
                    nc.scalar.activation(
                        y[:, rb - 1, :], flv[:, rb, WZ:, 1], ACTF.Identity,
                        bias=cC(rb), scale=R511,
                    )
                t0 = ptmp.tile([128, 3, NW], BF16, tag="t0")
                nc.scalar.activation(
                    t0[:], x[:], ACTF.Identity,
                    bias=cC(4 + 4 * b + 0), scale=cC(4 + 4 * b + 1),
                )
                t1 = ptmp.tile([128, 3, NW], BF16, tag="t1")
                nc.scalar.activation(
                    t1[:], x[:], ACTF.Identity,
                    bias=cC(4 + 4 * b + 2), scale=cC(4 + 4 * b + 3),
                )
                nc.vector.tensor_tensor(t1[:], y[:], t1[:], ALU.mult)
                nc.vector.tensor_tensor(t0[:], t0[:], t1[:], ALU.add)
                # masks (exact) on the ambiguous strips only
                nc.vector.scalar_tensor_tensor(
                    t0[:, :, 0:XSTRIP], x[:, :, 0:XSTRIP], 0.0,
                    t0[:, :, 0:XSTRIP], ALU.is_ge, ALU.mult,
                )
                nc.vector.scalar_tensor_tensor(
                    t0[:, 0:1, :], y[:, 0:1, :], 0.0, t0[:, 0:1, :],
                    ALU.is_ge, ALU.mult,
                )
                nc.vector.scalar_tensor_tensor(
                    t0[0:3, 1:2, :], y[0:3, 1:2, :], 0.0, t0[0:3, 1:2, :],
                    ALU.is_ge, ALU.mult,
                )
                nc.vector.tensor_tensor(
                    dtv[:, 1:4, WZ:], dtv[:, 1:4, WZ:], t0[:], ALU.add
                )

                # ---------------- flow update ------------------------------
                corr = ptmp.tile([128, FD * 2], BF16, tag="corr")
                corr2 = corr[:].rearrange("p (q c) -> p q c", c=2)
                nc.vector.tensor_tensor(corr2[:, :, 0], dt[:], g1[:], ALU.mult)
                nc.vector.tensor_tensor(corr2[:, :, 1], dt[:], g2[:], ALU.mult)
                nc.vector.tensor_tensor(fl[:], fl[:], corr[:], ALU.add)

                nc.sync.dma_start(
                    OUT[b].rearrange("(rb p) w c -> p rb w c", p=128), flv
                )
    legalize_single_wait(nc)
    return nc


# ---------------------------------------------------------------------------
# Post-pass: this walrus build encodes a single sync-wait slot per TPB
# instruction. Tile's sem assignment can emit 2+ waits on one instruction;
# hoist all but the last wait onto same-engine EventSemaphore carriers placed
# immediately before it (the sequencer then waits sequentially, which is
# semantically identical).
def legalize_single_wait(nc):
    import bass_rust

    capped = {
        mybir.EngineType.Activation,
        mybir.EngineType.DVE,
        mybir.EngineType.Pool,
        mybir.EngineType.PE,
        mybir.EngineType.SP,
    }
    exempt = {"EventSemaphore", "NoOp", "TriggerDma"}
    n = 0
    for fn in nc.m.functions:
        for blk in fn.blocks:
            insts = blk.instructions  # live list
            rebuilt = []
            changed = False
            for inst in list(insts):
                si = inst.sync_info
                waits = list(si.on_wait) if si is not None else []
                if (
                    len(waits) > 1
                    and inst.engine in capped
                    and str(inst.opcode) not in exempt
                ):
                    for w in waits[:-1]:
                        ev = mybir.InstEventSemaphore(
                            name=f"waitcarrier_{inst.name}_{n}", ins=[], outs=[]
                        )
                        ev.engine = inst.engine
                        ev.sync_info = bass_rust.SyncInfo(
                            on_wait=[w], on_update=[]
                        )
                        rebuilt.append(ev)
                        n += 1
                    inst.sync_info = bass_rust.SyncInfo(
                        on_wait=[waits[-1]], on_update=list(si.on_update)
                    )
                    changed = True
                rebuilt.append(inst)
            if changed:
                insts[:] = rebuilt
    return n


def host_consts(I1c: np.ndarray, n_imgs: int) -> np.ndarray:
    """[128, 4 + 4*n] f32: col rb: (2*(128*rb+p))*r511 - 1; then per-image
    -0.1-scaled bilinear coeffs A0,B0,A1,B1 of the corner patch."""
    f = np.float32
    cc = np.zeros((128, 4 + 4 * n_imgs), dtype=np.float32)
    p = np.arange(128, dtype=np.float64)
    for rb in range(4):
        cc[:, rb] = (2.0 * (128.0 * rb + p)) / 511.0 - 1.0
    for b in range(n_imgs):
        P = I1c[b, 0:3, 0:3].astype(np.float64)
        a0 = P[0, 0]
        b0 = P[0, 1] - P[0, 0]
        a1 = P[1, 0] - P[0, 0]
        b1 = P[1, 1] - P[1, 0] - P[0, 1] + P[0, 0]
        cc[:, 4 + 4 * b + 0] = f(-0.1 * a0)
        cc[:, 4 + 4 * b + 1] = f(-0.1 * b0)
        cc[:, 4 + 4 * b + 2] = f(-0.1 * a1)
        cc[:, 4 + 4 * b + 3] = f(-0.1 * b1)
    return cc


def host_gx() -> np.ndarray:
    w = np.arange(WZ, 512, dtype=np.float64)
    gx = (2.0 * w) / 511.0 - 1.0
    return np.tile(gx.astype(ml_dtypes.bfloat16), (128, 3))


_NC = None


def _get_nc():
    global _NC
    if _NC is None:
        _NC = build_nc(4)
    return _NC


def run(I1, I2, flow, trace=False, **kw):
    I1 = np.asarray(I1)
    I2 = np.asarray(I2)
    flow = np.asarray(flow)
    assert float(np.abs(flow).max()) < 6.0, "flow magnitude exceeds mask-strip bound"
    nc = _get_nc()
    gx = host_gx()
    in_maps = []
    per = I1.shape[0] // 8
    for c in range(8):
        sl = slice(c * per, (c + 1) * per)
        i1c = np.ascontiguousarray(I1[sl, :, :, 0], dtype=np.float32)
        in_maps.append(
            {
                "I1": i1c,
                "I2": np.ascontiguousarray(I2[sl, :, :, 0], dtype=np.float32),
                "FL": np.ascontiguousarray(flow[sl], dtype=np.float32),
                "GX": gx,
                "CC": host_consts(i1c, per),
            }
        )
    res = run_bass_kernel_spmd(nc, in_maps, list(range(8)), trace=trace, **kw)
    out = np.concatenate(
        [r["OUT"].astype(np.float32) for r in res.results], axis=0
    )
    return out, res


def kernel(I1, I2, flow):
    out, _ = run(I1, I2, flow)
    return out.astype(np.float32)
